# revision 13
# baseline (speedup 1.0000x reference)
"""Cross-attention Trainium2 Bass kernel (8 NeuronCores, SPMD).

Problem: B=4, Sd=Se=2048, E=1024, H=16, D=64 cross-attention
  Q = dec @ Wq; K = enc @ Wk; V = enc @ Wv
  out = softmax(Q K^T / sqrt(D)) V @ Wo + b_o

Sharding (hardcoded): core c -> batch b=c//2, head-group g=c%2 (8 heads).
Each core gets transposed bf16 activations (dec[b].T, enc[b].T) and its
column/row slice of the weights; returns per-PAIR partial outputs
out_t[pair] = (attn_pair @ Wo_pair)^T, [4, 1024, 2048] bf16. Host sums
the 8 partials per batch (f32) and adds the bias.

v2.2 design. ScalarE exp is the roofline (33.5M exps/core @128 lanes
@1.2GHz ~= 285us); TensorE work is ~277us — both must run ~100% busy:
- Scores S^T per head pair row-packed in the PE array (2x 64-contraction
  tiles), exp on ScalarE [128,1024] PSUM->SBUF, PV with a ones-column
  appended to V so softmax denominators fall out of the PV matmul.
- Emission is software-pipelined across sweeps: each sweep's last PVs +
  evac/normalize are carried as thunks into the next sweep's first
  chunks, and own-PVs lag scores by 7 chunks, so ScalarE never waits at
  sweep boundaries. kt/qt projection chains are woven between chunks.
- Normalization: denominator row -> bf16, rank-1 PE matmul (ones x row)
  broadcasts it across partitions into PSUM (~100 cycles), DVE fast
  approx reciprocal, multiply on gpsimd. No DRAM roundtrip.
- Output projection per (pair, q) right after that pair's normalize,
  woven into the next sweep; host sums partials. No end-of-kernel
  contraction over pairs -> ~no tail.
- Input DMAs split across sync/scalar/gpsimd queues; exp table
  preloaded at t=0.
"""

import numpy as np
import ml_dtypes
from contextlib import ExitStack

B = 4
SD = 2048
SE = 2048
E = 1024
H = 16
DH = 64
EL = 512          # local cols per core (8 heads)
NPAIR = 4         # head pairs per core
KCH = E // 128    # embed chunks (8)
SET = SE // 128   # se tiles (16)
SDQ = 512         # sd quarter
NQ = SD // SDQ    # 4
SCALE = 1.0 / np.sqrt(DH)
LAG = 7           # own-PV lags scores by this many chunks

_BUILT = None


def _build(debug=False):
    import concourse.bass as bass
    import concourse.tile as tile
    from concourse import bacc, mybir

    BF16 = mybir.dt.bfloat16
    F32 = mybir.dt.float32
    EXP = mybir.ActivationFunctionType.Exp

    nc = bacc.Bacc("TRN2", target_bir_lowering=False, debug=False)
    dec_t_d = nc.dram_tensor("dec_t", [E, SD], BF16, kind="ExternalInput").ap()
    enc_t_d = nc.dram_tensor("enc_t", [E, SE], BF16, kind="ExternalInput").ap()
    wq_d = nc.dram_tensor("wq", [E, EL], BF16, kind="ExternalInput").ap()
    wk_d = nc.dram_tensor("wk", [E, EL], BF16, kind="ExternalInput").ap()
    wv_d = nc.dram_tensor("wv", [E, EL], BF16, kind="ExternalInput").ap()
    wo_d = nc.dram_tensor("wo", [EL, E], BF16, kind="ExternalInput").ap()
    out_d = nc.dram_tensor("out_t", [NPAIR, E, SD], BF16,
                           kind="ExternalOutput").ap()
    dbg = {}
    if debug:
        dbg["kt"] = nc.dram_tensor("dbg_kt", [NPAIR, 128, SE], BF16, kind="ExternalOutput").ap()
        dbg["vp"] = nc.dram_tensor("dbg_vp", [SET, 128, 8 * 65], BF16, kind="ExternalOutput").ap()
        dbg["pt0"] = nc.dram_tensor("dbg_pt0", [128, 2 * SDQ], BF16, kind="ExternalOutput").ap()
        dbg["den"] = nc.dram_tensor("dbg_den", [1, 2 * SDQ], BF16, kind="ExternalOutput").ap()
        dbg["rbcr"] = nc.dram_tensor("dbg_rbcr", [128, SDQ], mybir.dt.float32, kind="ExternalOutput").ap()
        dbg["attn"] = nc.dram_tensor("dbg_attn", [NPAIR, 128, SDQ], BF16, kind="ExternalOutput").ap()

    with tile.TileContext(nc) as tc, ExitStack() as ctx:
        consts = ctx.enter_context(tc.tile_pool(name="consts", bufs=1))
        acts = ctx.enter_context(tc.tile_pool(name="acts", bufs=1))
        kt_pool = ctx.enter_context(tc.tile_pool(name="ktp", bufs=1))
        qt_pool = ctx.enter_context(tc.tile_pool(name="qtp", bufs=3))
        v_pool = ctx.enter_context(tc.tile_pool(name="vpool", bufs=1))
        pt_pool = ctx.enter_context(tc.tile_pool(name="pt", bufs=SET + LAG - 3))
        attn_pool = ctx.enter_context(tc.tile_pool(name="attn", bufs=2))
        den_pool = ctx.enter_context(tc.tile_pool(name="den", bufs=2))
        rbc_pool = ctx.enter_context(tc.tile_pool(name="rbc", bufs=2))
        evac = ctx.enter_context(tc.tile_pool(name="evac", bufs=4))
        ps_s = ctx.enter_context(tc.tile_pool(name="ps_s", bufs=2, space="PSUM"))
        ps_pv = ctx.enter_context(tc.tile_pool(name="ps_pv", bufs=2, space="PSUM"))
        ps_proj = ctx.enter_context(tc.tile_pool(name="ps_proj", bufs=2, space="PSUM"))

        # exp table preload: tiny activation at t=0 so the ~2.7us
        # ACT_TABLE_LOAD doesn't delay the first real exp.
        warm = consts.tile([128, 8], F32, tag="warm", name="warm")
        nc.vector.memset(warm[:], 0.0)
        nc.scalar.activation(warm[:], warm[:], EXP, scale=1.0)

        # ones for the rank-1 denominator broadcast
        ones_sb = consts.tile([65, 64], BF16, tag="ones", name="ones_sb")
        nc.vector.memset(ones_sb[:], 1.0)

        # ---------------- input DMAs (order = priority) ----------------
        enc_sb = [acts.tile([128, SE], BF16, tag=f"enc{k}", name=f"enc{k}")
                  for k in range(KCH)]
        dec_sb = [acts.tile([128, SD], BF16, tag=f"dec{k}", name=f"dec{k}")
                  for k in range(KCH)]
        wk_sb = consts.tile([128, KCH, EL], BF16, tag="wk", name="wk_sb")
        wq_sb = consts.tile([128, KCH, EL], BF16, tag="wq", name="wq_sb")
        wv_sb = consts.tile([128, KCH, EL], BF16, tag="wv", name="wv_sb")
        wo_sb = [consts.tile([128, E], BF16, tag=f"wo{p}", name=f"wo{p}")
                 for p in range(NPAIR)]
        # kt(p0) needs enc+wk; first qt needs dec[:, q0]+wq; vp needs wv.
        # Split across queues so issue overhead doesn't serialize.
        for k in range(KCH):
            eng = nc.sync if k % 2 == 0 else nc.scalar
            eng.dma_start(out=enc_sb[k][:], in_=enc_t_d[k * 128:(k + 1) * 128, :])
        for k in range(KCH):
            eng = nc.sync if k % 2 == 0 else nc.scalar
            eng.dma_start(out=wk_sb[:, k, :], in_=wk_d[k * 128:(k + 1) * 128, :])
        for k in range(KCH):
            eng = nc.sync if k % 2 == 0 else nc.scalar
            eng.dma_start(out=dec_sb[k][:, 0:SDQ],
                          in_=dec_t_d[k * 128:(k + 1) * 128, 0:SDQ])
        for k in range(KCH):
            eng = nc.sync if k % 2 == 0 else nc.scalar
            eng.dma_start(out=wq_sb[:, k, :], in_=wq_d[k * 128:(k + 1) * 128, :])
        for k in range(KCH):
            eng = nc.sync if k % 2 == 0 else nc.scalar
            eng.dma_start(out=wv_sb[:, k, :], in_=wv_d[k * 128:(k + 1) * 128, :])
        for k in range(KCH):
            nc.gpsimd.dma_start(out=dec_sb[k][:, SDQ:SD],
                                in_=dec_t_d[k * 128:(k + 1) * 128, SDQ:SD])
        for p in range(NPAIR):
            nc.gpsimd.dma_start(out=wo_sb[p][:], in_=wo_d[p * 128:(p + 1) * 128, :])

        # persistent sbuf tensors
        # V' layout: per head 65 cols = [V_h (64) | ones]; ones column makes
        # the PV matmul emit softmax denominators for free (row 64 of out)
        vp = [v_pool.tile([128, 8 * 65], BF16, tag=f"vp{i}", name=f"vp{i}")
              for i in range(SET)]
        kt = [kt_pool.tile([128, SE], BF16, tag=f"kt{p}", name=f"kt{p}")
              for p in range(NPAIR)]

        # ---------------- projection chain helpers ----------------
        def emit_kt_chain(pair, n):
            ps = ps_proj.tile([128, 512], F32, tag="ps_proj", name="ps_proj")
            for k in range(KCH):
                nc.tensor.matmul(
                    ps[:], wk_sb[:, k, pair * 128:(pair + 1) * 128],
                    enc_sb[k][:, n * 512:(n + 1) * 512],
                    start=(k == 0), stop=(k == KCH - 1))
            nc.vector.tensor_copy(kt[pair][:, n * 512:(n + 1) * 512], ps[:])

        qts = {}

        def emit_qt(pair, q):
            qt = qt_pool.tile([128, SDQ], BF16, tag="qt", name="qt")
            ps = ps_proj.tile([128, 512], F32, tag="ps_proj", name="ps_proj")
            for k in range(KCH):
                nc.tensor.matmul(
                    ps[:], wq_sb[:, k, pair * 128:(pair + 1) * 128],
                    dec_sb[k][:, q * SDQ:(q + 1) * SDQ],
                    start=(k == 0), stop=(k == KCH - 1))
            nc.vector.tensor_copy(qt[:], ps[:])
            qts[(pair, q)] = qt

        def emit_v_chain(m):
            ps = ps_proj.tile([128, 512], F32, tag="ps_proj", name="ps_proj")
            for k in range(KCH):
                nc.tensor.matmul(
                    ps[:], enc_sb[k][:, m * 128:(m + 1) * 128], wv_sb[:, k, :],
                    start=(k == 0), stop=(k == KCH - 1))
            vsrc = ps[:, :].rearrange("p (h d) -> p h d", h=8)
            vdst = vp[m].rearrange("p (h d) -> p h d", d=65)
            nc.vector.tensor_copy(vdst[:, :, 0:64], vsrc)
            nc.vector.memset(vdst[:, :, 64:65], 1.0)

        # ---------------- per-(pair,q) output projection ----------------
        attn_tiles = {}

        def outproj_chain(pair, q, m0, nm):
            """Partial output rows m0..m0+nm for this pair's q-block."""
            at = attn_tiles[(pair, q)]
            for m in range(m0, m0 + nm):
                ps = ps_proj.tile([128, 512], F32, tag="ps_proj", name="ps_proj")
                nc.tensor.matmul(ps[:], wo_sb[pair][:, m * 128:(m + 1) * 128],
                                 at[:, :], start=True, stop=True)
                ot = evac.tile([128, 512], BF16, tag="ot", name="ot")
                nc.vector.tensor_copy(ot[:], ps[:])
                nc.sync.dma_start(
                    out=out_d[pair, m * 128:(m + 1) * 128,
                              q * SDQ:(q + 1) * SDQ],
                    in_=ot[:])

        # ---------------- attention sweep for one (pair, q) -------------
        def make_evac_norm(pair, q, pvs):
            def evac_norm():
                at = attn_pool.tile([128, SDQ], BF16, tag=f"attn{pair}",
                                    name=f"attn{pair}")
                attn_tiles[(pair, q)] = at
                den = den_pool.tile([65, 2 * SDQ], BF16, tag="den", name="den")
                for h in range(2):
                    nc.vector.tensor_copy(at[h * 64:(h + 1) * 64, :],
                                          pvs[h][0:64, :])
                    nc.vector.tensor_copy(den[64:65, h * SDQ:(h + 1) * SDQ],
                                          pvs[h][64:65, :])
                # rank-1 broadcast: psum[0:64] = d_h0 rows, [64:128] = d_h1
                bps = ps_proj.tile([128, 512], F32, tag="ps_proj",
                                   name="bps")
                for h in range(2):
                    nc.tensor.matmul(
                        bps[h * 64:(h + 1) * 64, :], ones_sb[64:65, :],
                        den[64:65, h * SDQ:(h + 1) * SDQ],
                        start=True, stop=True, tile_position=(64, h * 64))
                rbcf = rbc_pool.tile([128, SDQ], F32, tag="rbcf", name="rbcf")
                nc.vector.tensor_copy(rbcf[:], bps[:])
                rbcr = rbc_pool.tile([128, SDQ], F32, tag="rbcr", name="rbcr")
                nc.vector.reciprocal_approx_fast(rbcr[:], rbcf[:])
                rbcb = rbc_pool.tile([128, SDQ], BF16, tag="rbcb", name="rbcb")
                nc.vector.tensor_copy(rbcb[:], rbcr[:])
                if debug and (pair, q) == (0, 0):
                    nc.sync.dma_start(out=dbg["den"][:], in_=den[64:65, :])
                    nc.sync.dma_start(out=dbg["rbcr"][:], in_=rbcr[:])
                nc.gpsimd.tensor_mul(at[:], at[:], rbcb[:])
                if debug and q == 0:
                    nc.sync.dma_start(out=dbg["attn"][pair], in_=at[:])
            return evac_norm

        def sweep(pair, q, prev_tail, woven=(), defer_pv=False):
            """16 score chunks -> exp -> PV. prev_tail: thunks from the
            previous sweep (its last PVs + evac/norm), consumed 2/chunk in
            the first chunks. woven: extra PE work (projection chains),
            spread across later chunks. Returns this sweep's tail."""
            woven = list(woven)
            tail = list(prev_tail)
            qt = qts.pop((pair, q))
            pvs = [ps_pv.tile([65, SDQ], F32, tag="ps_pv", name=f"ps_pv{h}")
                   for h in range(2)]
            pts = {}

            def pv2(i):
                pt = pts.pop(i)
                first, last = (i == 0), (i == SET - 1)
                for h in range(2):
                    hl = pair * 2 + h
                    nc.tensor.matmul(
                        pvs[h][:, :],
                        vp[i][:, hl * 65:hl * 65 + 65],
                        pt[:, h * SDQ:(h + 1) * SDQ],
                        start=first, stop=last)

            n_w = len(woven)
            w_start = 4  # don't weave before chunk 4 (tail occupies 0..3)
            interval = max(1, (SET - w_start) // n_w) if n_w else SET + 1
            for i in range(SET):
                sp = ps_s.tile([128, 2 * SDQ], F32, tag="ps_s", name="ps_s")
                for h in range(2):
                    nc.tensor.matmul(
                        sp[:, h * SDQ:(h + 1) * SDQ],
                        kt[pair][h * 64:(h + 1) * 64, i * 128:(i + 1) * 128],
                        qt[h * 64:(h + 1) * 64, :],
                        start=True, stop=True,
                        tile_position=(h * 64, 0))
                pt = pt_pool.tile([128, 2 * SDQ], BF16, tag="pt", name="pt")
                nc.scalar.activation(pt[:], sp[:], EXP, scale=float(SCALE))
                pts[i] = pt
                if debug and (pair, q, i) == (0, 0, 0):
                    nc.sync.dma_start(out=dbg["pt0"][:], in_=pt[:])
                for _ in range(2):      # previous sweep's tail, 2 per chunk
                    if tail:
                        tail.pop(0)()
                if not defer_pv and i >= LAG:
                    pv2(i - LAG)
                if woven and i >= w_start and (i - w_start) % interval == interval - 1:
                    woven.pop(0)()
            while tail:
                tail.pop(0)()
            while woven:
                woven.pop(0)()
            if defer_pv:
                for i in range(SET):
                    pv2(i)
                my_tail = [make_evac_norm(pair, q, pvs)]
            else:
                my_tail = [(lambda i=i: pv2(i)) for i in range(SET - LAG, SET)]
                my_tail.append(make_evac_norm(pair, q, pvs))
            return my_tail

        # ---------------- emission schedule ----------------
        for n in range(NQ):
            emit_kt_chain(0, n)
        emit_qt(0, 0)

        order = [(p, q) for p in range(NPAIR) for q in range(NQ)]
        tail = []
        for j, (pair, q) in enumerate(order):
            woven = []
            if (pair, q) == (0, 0):
                woven += [(lambda m=m: emit_v_chain(m)) for m in range(SET)]
            if j + 1 < len(order):
                np_, nq_ = order[j + 1]
                woven.insert(0, lambda p=np_, qq=nq_: emit_qt(p, qq))
            if q in (1, 2) and pair + 1 < NPAIR:
                ns = (0, 1) if q == 1 else (2, 3)
                woven += [(lambda n=n, p=pair + 1: emit_kt_chain(p, n))
                          for n in ns]
            if j >= 1:
                pp, pq = order[j - 1]
                woven += [(lambda m0=m0, p=pp, qq=pq: outproj_chain(p, qq, m0, 2))
                          for m0 in range(0, KCH, 2)]
            tail = sweep(pair, q, tail, woven=woven,
                         defer_pv=(pair, q) == (0, 0))
        # flush the last sweep's tail + its output projection
        while tail:
            tail.pop(0)()
        for m0 in range(0, KCH, 2):
            outproj_chain(NPAIR - 1, NQ - 1, m0, 2)
        if debug:
            for p in range(NPAIR):
                nc.sync.dma_start(out=dbg["kt"][p], in_=kt[p][:])
            for i in range(SET):
                nc.sync.dma_start(out=dbg["vp"][i], in_=vp[i][:])

    nc.compile()
    return nc


def _get_built():
    global _BUILT
    if _BUILT is None:
        _BUILT = _build()
    return _BUILT


def kernel(decoder_input, encoder_output, W_q, W_k, W_v, W_o, b_o):
    from concourse.bass_utils import run_bass_kernel_spmd

    dec = np.asarray(decoder_input, dtype=np.float32)
    enc = np.asarray(encoder_output, dtype=np.float32)
    W_q = np.asarray(W_q, dtype=np.float32)
    W_k = np.asarray(W_k, dtype=np.float32)
    W_v = np.asarray(W_v, dtype=np.float32)
    W_o = np.asarray(W_o, dtype=np.float32)
    b_o = np.asarray(b_o, dtype=np.float32)

    bf = lambda a: np.ascontiguousarray(a).astype(ml_dtypes.bfloat16)

    nc = _get_built()
    in_maps = []
    for c in range(8):
        b, g = divmod(c, 2)
        sl = slice(g * EL, (g + 1) * EL)
        in_maps.append({
            "dec_t": bf(dec[b].T),
            "enc_t": bf(enc[b].T),
            "wq": bf(W_q[:, sl]),
            "wk": bf(W_k[:, sl]),
            "wv": bf(W_v[:, sl]),
            "wo": bf(W_o[sl, :]),
        })
    res = run_bass_kernel_spmd(nc, in_maps, core_ids=list(range(8)))
    out = np.empty((B, SD, E), np.float32)
    for b in range(B):
        acc = res.results[2 * b]["out_t"].astype(np.float32).sum(axis=0)
        acc += res.results[2 * b + 1]["out_t"].astype(np.float32).sum(axis=0)
        out[b] = acc.T
        out[b] += b_o
    return out


if __name__ == "__main__":
    _get_built()
    print("kernel built OK")


# revision 20
# speedup vs baseline: 1.0040x; 1.0040x over previous
"""Cross-attention Trainium2 Bass kernel (8 NeuronCores, SPMD).

Problem: B=4, Sd=Se=2048, E=1024, H=16, D=64 cross-attention
  Q = dec @ Wq; K = enc @ Wk; V = enc @ Wv
  out = softmax(Q K^T / sqrt(D)) V @ Wo + b_o

Sharding (hardcoded): core c -> batch b=c//2, head-group g=c%2 (8 heads).
Each core gets transposed bf16 activations (dec[b].T, enc[b].T) and its
column/row slice of the weights; returns per-PAIR partial outputs
out_t[pair] = (attn_pair @ Wo_pair)^T, [4, 1024, 2048] bf16. Host sums
the 8 partials per batch (f32) and adds the bias.

v2.2 design. ScalarE exp is the roofline (33.5M exps/core @128 lanes
@1.2GHz ~= 285us); TensorE work is ~277us — both must run ~100% busy:
- Scores S^T per head pair row-packed in the PE array (2x 64-contraction
  tiles), exp on ScalarE [128,1024] PSUM->SBUF, PV with a ones-column
  appended to V so softmax denominators fall out of the PV matmul.
- Emission is software-pipelined across sweeps: each sweep's last PVs +
  evac/normalize are carried as thunks into the next sweep's first
  chunks, and own-PVs lag scores by 7 chunks, so ScalarE never waits at
  sweep boundaries. kt/qt projection chains are woven between chunks.
- Normalization: denominator row -> bf16, rank-1 PE matmul (ones x row)
  broadcasts it across partitions into PSUM (~100 cycles), DVE fast
  approx reciprocal, multiply on gpsimd. No DRAM roundtrip.
- Output projection per (pair, q) right after that pair's normalize,
  woven into the next sweep; host sums partials. No end-of-kernel
  contraction over pairs -> ~no tail.
- Input DMAs split across sync/scalar/gpsimd queues; exp table
  preloaded at t=0.
"""

import numpy as np
import ml_dtypes
from contextlib import ExitStack

B = 4
SD = 2048
SE = 2048
E = 1024
H = 16
DH = 64
EL = 512          # local cols per core (8 heads)
NPAIR = 4         # head pairs per core
KCH = E // 128    # embed chunks (8)
SET = SE // 128   # se tiles (16)
SDQ = 512         # sd quarter
NQ = SD // SDQ    # 4
SCALE = 1.0 / np.sqrt(DH)
LAG = 7           # own-PV lags scores by this many chunks
# Schraudolph exp constants (bf16-bits domain), with the 1/8 score scale
# folded into the multiplier. Validated on HW: matches the round-model.
EXP_A16 = 12102203.1614 / 65536.0 * SCALE
EXP_B16 = (1065353216.0 - 486411.0) / 65536.0
DVE_EXP_CHUNKS = (3, 8, 13)   # 3/16 of exps go to the vector engine

_BUILT = None


def _build(debug=False):
    import concourse.bass as bass
    import concourse.tile as tile
    from concourse import bacc, mybir

    BF16 = mybir.dt.bfloat16
    F32 = mybir.dt.float32
    EXP = mybir.ActivationFunctionType.Exp

    nc = bacc.Bacc("TRN2", target_bir_lowering=False, debug=False)
    dec_t_d = nc.dram_tensor("dec_t", [E, SD], BF16, kind="ExternalInput").ap()
    enc_t_d = nc.dram_tensor("enc_t", [E, SE], BF16, kind="ExternalInput").ap()
    wq_d = nc.dram_tensor("wq", [E, EL], BF16, kind="ExternalInput").ap()
    wk_d = nc.dram_tensor("wk", [E, EL], BF16, kind="ExternalInput").ap()
    wv_d = nc.dram_tensor("wv", [E, EL], BF16, kind="ExternalInput").ap()
    wo_d = nc.dram_tensor("wo", [EL, E], BF16, kind="ExternalInput").ap()
    out_d = nc.dram_tensor("out_t", [E, SD], BF16,
                           kind="ExternalOutput").ap()
    dbg = {}
    if debug:
        dbg["kt"] = nc.dram_tensor("dbg_kt", [NPAIR, 128, SE], BF16, kind="ExternalOutput").ap()
        dbg["vp"] = nc.dram_tensor("dbg_vp", [SET, 128, 8 * 65], BF16, kind="ExternalOutput").ap()
        dbg["pt0"] = nc.dram_tensor("dbg_pt0", [128, 2 * SDQ], BF16, kind="ExternalOutput").ap()
        dbg["den"] = nc.dram_tensor("dbg_den", [1, 2 * SDQ], BF16, kind="ExternalOutput").ap()
        dbg["rbcr"] = nc.dram_tensor("dbg_rbcr", [128, SDQ], mybir.dt.float32, kind="ExternalOutput").ap()
        dbg["attn"] = nc.dram_tensor("dbg_attn", [NPAIR, 128, SDQ], BF16, kind="ExternalOutput").ap()

    with tile.TileContext(nc) as tc, ExitStack() as ctx:
        consts = ctx.enter_context(tc.tile_pool(name="consts", bufs=1))
        acts = ctx.enter_context(tc.tile_pool(name="acts", bufs=1))
        kt_pool = ctx.enter_context(tc.tile_pool(name="ktp", bufs=1))
        qt_pool = ctx.enter_context(tc.tile_pool(name="qtp", bufs=3))
        v_pool = ctx.enter_context(tc.tile_pool(name="vpool", bufs=1))
        pt_pool = ctx.enter_context(tc.tile_pool(name="pt", bufs=SET + 2))
        attn_pool = ctx.enter_context(tc.tile_pool(name="attn", bufs=4))
        den_pool = ctx.enter_context(tc.tile_pool(name="den", bufs=2))
        rbc_pool = ctx.enter_context(tc.tile_pool(name="rbc", bufs=2))
        evac = ctx.enter_context(tc.tile_pool(name="evac", bufs=4))
        ps_s = ctx.enter_context(tc.tile_pool(name="ps_s", bufs=2, space="PSUM"))
        ps_pv = ctx.enter_context(tc.tile_pool(name="ps_pv", bufs=2, space="PSUM"))
        ps_proj = ctx.enter_context(tc.tile_pool(name="ps_proj", bufs=2, space="PSUM"))

        # exp table preload: tiny activation at t=0 so the ~2.7us
        # ACT_TABLE_LOAD doesn't delay the first real exp.
        warm = consts.tile([128, 8], F32, tag="warm", name="warm")
        nc.vector.memset(warm[:], 0.0)
        nc.scalar.activation(warm[:], warm[:], EXP, scale=1.0)

        # ones for the rank-1 denominator broadcast
        ones_sb = consts.tile([65, 64], BF16, tag="ones", name="ones_sb")
        nc.vector.memset(ones_sb[:], 1.0)

        # ---------------- input DMAs (order = priority) ----------------
        enc_sb = [acts.tile([128, SE], BF16, tag=f"enc{k}", name=f"enc{k}")
                  for k in range(KCH)]
        dec_sb = [acts.tile([128, SD], BF16, tag=f"dec{k}", name=f"dec{k}")
                  for k in range(KCH)]
        wk_sb = consts.tile([128, KCH, EL], BF16, tag="wk", name="wk_sb")
        wq_sb = consts.tile([128, KCH, EL], BF16, tag="wq", name="wq_sb")
        wv_sb = consts.tile([128, KCH, EL], BF16, tag="wv", name="wv_sb")
        wo_sb = [consts.tile([128, E], BF16, tag=f"wo{p}", name=f"wo{p}")
                 for p in range(NPAIR)]
        # kt(p0) needs enc+wk; first qt needs dec[:, q0]+wq; vp needs wv.
        # Split across queues so issue overhead doesn't serialize.
        for k in range(KCH):
            eng = nc.sync if k % 2 == 0 else nc.scalar
            eng.dma_start(out=enc_sb[k][:], in_=enc_t_d[k * 128:(k + 1) * 128, :])
        for k in range(KCH):
            eng = nc.sync if k % 2 == 0 else nc.scalar
            eng.dma_start(out=wk_sb[:, k, :], in_=wk_d[k * 128:(k + 1) * 128, :])
        for k in range(KCH):
            eng = nc.sync if k % 2 == 0 else nc.scalar
            eng.dma_start(out=dec_sb[k][:, 0:SDQ],
                          in_=dec_t_d[k * 128:(k + 1) * 128, 0:SDQ])
        for k in range(KCH):
            eng = nc.sync if k % 2 == 0 else nc.scalar
            eng.dma_start(out=wq_sb[:, k, :], in_=wq_d[k * 128:(k + 1) * 128, :])
        for k in range(KCH):
            eng = nc.sync if k % 2 == 0 else nc.scalar
            eng.dma_start(out=wv_sb[:, k, :], in_=wv_d[k * 128:(k + 1) * 128, :])
        for k in range(KCH):
            nc.gpsimd.dma_start(out=dec_sb[k][:, SDQ:SD],
                                in_=dec_t_d[k * 128:(k + 1) * 128, SDQ:SD])
        for p in range(NPAIR):
            nc.gpsimd.dma_start(out=wo_sb[p][:], in_=wo_d[p * 128:(p + 1) * 128, :])

        # persistent sbuf tensors
        # V' layout: per head 65 cols = [V_h (64) | ones]; ones column makes
        # the PV matmul emit softmax denominators for free (row 64 of out)
        vp = [v_pool.tile([128, 8 * 65], BF16, tag=f"vp{i}", name=f"vp{i}")
              for i in range(SET)]
        kt = [kt_pool.tile([128, SE], BF16, tag=f"kt{p}", name=f"kt{p}")
              for p in range(NPAIR)]

        # ---------------- projection chain helpers ----------------
        def emit_kt_chain(pair, n):
            ps = ps_proj.tile([128, 512], F32, tag="ps_proj", name="ps_proj")
            for k in range(KCH):
                nc.tensor.matmul(
                    ps[:], wk_sb[:, k, pair * 128:(pair + 1) * 128],
                    enc_sb[k][:, n * 512:(n + 1) * 512],
                    start=(k == 0), stop=(k == KCH - 1))
            nc.vector.tensor_copy(kt[pair][:, n * 512:(n + 1) * 512], ps[:])

        qts = {}

        def emit_qt(pair, q):
            qt = qt_pool.tile([128, SDQ], BF16, tag="qt", name="qt")
            ps = ps_proj.tile([128, 512], F32, tag="ps_proj", name="ps_proj")
            for k in range(KCH):
                nc.tensor.matmul(
                    ps[:], wq_sb[:, k, pair * 128:(pair + 1) * 128],
                    dec_sb[k][:, q * SDQ:(q + 1) * SDQ],
                    start=(k == 0), stop=(k == KCH - 1))
            nc.vector.tensor_copy(qt[:], ps[:])
            qts[(pair, q)] = qt

        def emit_v_chain(m):
            ps = ps_proj.tile([128, 512], F32, tag="ps_proj", name="ps_proj")
            for k in range(KCH):
                nc.tensor.matmul(
                    ps[:], enc_sb[k][:, m * 128:(m + 1) * 128], wv_sb[:, k, :],
                    start=(k == 0), stop=(k == KCH - 1))
            vsrc = ps[:, :].rearrange("p (h d) -> p h d", h=8)
            vdst = vp[m].rearrange("p (h d) -> p h d", d=65)
            nc.vector.tensor_copy(vdst[:, :, 0:64], vsrc)
            nc.vector.memset(vdst[:, :, 64:65], 1.0)

        # ---------------- per-(pair,q) output projection ----------------
        attn_tiles = {}

        def outproj_chain(q, m):
            """Output rows m*128.. for q-block, contracted over all pairs."""
            ps = ps_proj.tile([128, 512], F32, tag="ps_proj", name="ps_proj")
            for p in range(NPAIR):
                nc.tensor.matmul(ps[:], wo_sb[p][:, m * 128:(m + 1) * 128],
                                 attn_tiles[(p, q)][:, :],
                                 start=(p == 0), stop=(p == NPAIR - 1))
            ot = evac.tile([128, 512], BF16, tag="ot", name="ot")
            nc.vector.tensor_copy(ot[:], ps[:])
            nc.sync.dma_start(
                out=out_d[m * 128:(m + 1) * 128, q * SDQ:(q + 1) * SDQ],
                in_=ot[:])

        # ---------------- attention sweep for one (pair, q) -------------
        def make_evac_norm(pair, q, pvs):
            def evac_norm():
                at = attn_pool.tile([128, SDQ], BF16, tag=f"attn{pair}",
                                    name=f"attn{pair}")
                attn_tiles[(pair, q)] = at
                den = den_pool.tile([65, 2 * SDQ], BF16, tag="den", name="den")
                for h in range(2):
                    nc.vector.tensor_copy(at[h * 64:(h + 1) * 64, :],
                                          pvs[h][0:64, :])
                    nc.vector.tensor_copy(den[64:65, h * SDQ:(h + 1) * SDQ],
                                          pvs[h][64:65, :])
                # rank-1 broadcast: psum[0:64] = d_h0 rows, [64:128] = d_h1
                bps = ps_proj.tile([128, 512], F32, tag="ps_proj",
                                   name="bps")
                for h in range(2):
                    nc.tensor.matmul(
                        bps[h * 64:(h + 1) * 64, :], ones_sb[64:65, :],
                        den[64:65, h * SDQ:(h + 1) * SDQ],
                        start=True, stop=True, tile_position=(64, h * 64))
                rbcf = rbc_pool.tile([128, SDQ], F32, tag="rbcf", name="rbcf")
                nc.vector.tensor_copy(rbcf[:], bps[:])
                rbcr = rbc_pool.tile([128, SDQ], F32, tag="rbcr", name="rbcr")
                nc.vector.reciprocal_approx_fast(rbcr[:], rbcf[:])
                rbcb = rbc_pool.tile([128, SDQ], BF16, tag="rbcb", name="rbcb")
                nc.vector.tensor_copy(rbcb[:], rbcr[:])
                if debug and (pair, q) == (0, 0):
                    nc.sync.dma_start(out=dbg["den"][:], in_=den[64:65, :])
                    nc.sync.dma_start(out=dbg["rbcr"][:], in_=rbcr[:])
                nc.gpsimd.tensor_mul(at[:], at[:], rbcb[:])
                if debug and q == 0:
                    nc.sync.dma_start(out=dbg["attn"][pair], in_=at[:])
            return evac_norm

        def sweep(pair, q, prev_tail, woven=(), defer_pv=False):
            """16 score chunks -> exp -> PV. prev_tail: thunks from the
            previous sweep (its last PVs + evac/norm), consumed 2/chunk in
            the first chunks. woven: extra PE work (projection chains),
            spread across later chunks. Returns this sweep's tail."""
            woven = list(woven)
            tail = list(prev_tail)
            qt = qts.pop((pair, q))
            pvs = [ps_pv.tile([65, SDQ], F32, tag="ps_pv", name=f"ps_pv{h}")
                   for h in range(2)]
            pts = {}

            def pv2(i):
                pt = pts.pop(i)
                first, last = (i == 0), (i == SET - 1)
                for h in range(2):
                    hl = pair * 2 + h
                    nc.tensor.matmul(
                        pvs[h][:, :],
                        vp[i][:, hl * 65:hl * 65 + 65],
                        pt[:, h * SDQ:(h + 1) * SDQ],
                        start=first, stop=last)

            n_w = len(woven)
            w_start = 4  # don't weave before chunk 4 (tail occupies 0..3)
            interval = max(1, (SET - w_start) // n_w) if n_w else SET + 1
            for i in range(SET):
                sp = ps_s.tile([128, 2 * SDQ], F32, tag="ps_s", name="ps_s")
                for h in range(2):
                    nc.tensor.matmul(
                        sp[:, h * SDQ:(h + 1) * SDQ],
                        kt[pair][h * 64:(h + 1) * 64, i * 128:(i + 1) * 128],
                        qt[h * 64:(h + 1) * 64, :],
                        start=True, stop=True,
                        tile_position=(h * 64, 0))
                pt = pt_pool.tile([128, 2 * SDQ], BF16, tag="pt", name="pt")
                if i in DVE_EXP_CHUNKS:
                    # Schraudolph exp on DVE (offloads ScalarE, the roofline):
                    # bf16 bits of e^(x/8) ~= round(x*A + B) as int16
                    nc.vector.tensor_scalar(
                        pt[:].bitcast(mybir.dt.int16), sp[:],
                        EXP_A16, EXP_B16,
                        mybir.AluOpType.mult, mybir.AluOpType.add)
                else:
                    nc.scalar.activation(pt[:], sp[:], EXP, scale=float(SCALE))
                pts[i] = pt
                if debug and (pair, q, i) == (0, 0, 0):
                    nc.sync.dma_start(out=dbg["pt0"][:], in_=pt[:])
                for _ in range(2):      # previous sweep's tail, 2 per chunk
                    if tail:
                        tail.pop(0)()
                if not defer_pv and i >= LAG:
                    pv2(i - LAG)
                if woven and i >= w_start and (i - w_start) % interval == interval - 1:
                    woven.pop(0)()
            while tail:
                tail.pop(0)()
            while woven:
                woven.pop(0)()
            if defer_pv:
                for i in range(SET):
                    pv2(i)
                my_tail = [make_evac_norm(pair, q, pvs)]
            else:
                my_tail = [(lambda i=i: pv2(i)) for i in range(SET - LAG, SET)]
                my_tail.append(make_evac_norm(pair, q, pvs))
            return my_tail

        # ---------------- emission schedule ----------------
        for n in range(NQ):
            emit_kt_chain(0, n)
        emit_qt(0, 0)

        order = [(p, q) for p in range(NPAIR) for q in range(NQ)]
        tail = []
        for j, (pair, q) in enumerate(order):
            woven = []
            if (pair, q) == (0, 0):
                woven += [(lambda m=m: emit_v_chain(m)) for m in range(SET)]
            if j + 1 < len(order):
                np_, nq_ = order[j + 1]
                woven.insert(0, lambda p=np_, qq=nq_: emit_qt(p, qq))
            if q in (1, 2) and pair + 1 < NPAIR:
                ns = (0, 1) if q == 1 else (2, 3)
                woven += [(lambda n=n, p=pair + 1: emit_kt_chain(p, n))
                          for n in ns]
            if pair == 3 and q >= 1:
                woven += [(lambda m=m, qq=q - 1: outproj_chain(qq, m))
                          for m in range(KCH)]
            tail = sweep(pair, q, tail, woven=woven,
                         defer_pv=(pair, q) == (0, 0))
        # flush the last sweep's tail + the last q-block's output projection
        while tail:
            tail.pop(0)()
        for m in range(KCH):
            outproj_chain(NQ - 1, m)
        if debug:
            for p in range(NPAIR):
                nc.sync.dma_start(out=dbg["kt"][p], in_=kt[p][:])
            for i in range(SET):
                nc.sync.dma_start(out=dbg["vp"][i], in_=vp[i][:])

    nc.compile()
    return nc


def _get_built():
    global _BUILT
    if _BUILT is None:
        _BUILT = _build()
    return _BUILT


def kernel(decoder_input, encoder_output, W_q, W_k, W_v, W_o, b_o):
    from concourse.bass_utils import run_bass_kernel_spmd

    dec = np.asarray(decoder_input, dtype=np.float32)
    enc = np.asarray(encoder_output, dtype=np.float32)
    W_q = np.asarray(W_q, dtype=np.float32)
    W_k = np.asarray(W_k, dtype=np.float32)
    W_v = np.asarray(W_v, dtype=np.float32)
    W_o = np.asarray(W_o, dtype=np.float32)
    b_o = np.asarray(b_o, dtype=np.float32)

    bf = lambda a: np.ascontiguousarray(a).astype(ml_dtypes.bfloat16)

    nc = _get_built()
    in_maps = []
    for c in range(8):
        b, g = divmod(c, 2)
        sl = slice(g * EL, (g + 1) * EL)
        in_maps.append({
            "dec_t": bf(dec[b].T),
            "enc_t": bf(enc[b].T),
            "wq": bf(W_q[:, sl]),
            "wk": bf(W_k[:, sl]),
            "wv": bf(W_v[:, sl]),
            "wo": bf(W_o[sl, :]),
        })
    res = run_bass_kernel_spmd(nc, in_maps, core_ids=list(range(8)))
    out = np.empty((B, SD, E), np.float32)
    for b in range(B):
        out[b] = (res.results[2 * b]["out_t"].astype(np.float32)
                  + res.results[2 * b + 1]["out_t"].astype(np.float32)).T
        out[b] += b_o
    return out


if __name__ == "__main__":
    _get_built()
    print("kernel built OK")
